# revision 10
# baseline (speedup 1.0000x reference)
"""Trainium2 Bass kernel for nn_DigitConvolutionalModel.

Model: x[B,784] -> conv3x3(valid, 28x28->26x26) -> flatten -> Linear(676,256)
       -> relu -> Linear(256,10).

The conv is linear, so it is folded into the first Linear on the host:
  h_pre = x @ W1eff + b1,  W1eff[784,256] = C @ W1.T  (C = conv as matrix)
leaving a plain 2-layer MLP for the device:
  out = relu(x @ W1eff + b1) @ W2.T + b2

Sharding: pure data parallelism over the batch dim across 8 NeuronCores
(8192 samples/core); weights replicated. Compute in bf16 with fp32 PSUM
accumulation. x is transposed on the host so the contraction dim (784,
zero-padded to 7*128) lands on SBUF partitions.

Schedule (per core):
- All of x stays SBUF-resident (114KB/partition); every load is pre-issued
  up front so DMA runs ahead of the PE with no buffer-recycle stalls.
- The sync queue carries the big transfers in exact need-order (w1 chunk 0,
  x group-0 quarter 0, w1 rest, remaining quarters, ...). Group 0 is
  stored quarter-major and group 1 half-major on the host so these partial
  loads are contiguous per partition and move at full HBM bandwidth.
- A warm-up matmul burst bridges the PE from program start to first data
  so the HAM clock gate reaches 8/8 with minimal cold-rate real work.
- Layer 2 (256->10) is packed 4 groups wide into PE column groups via
  tile_position, cutting its PE stream time ~4x.
- The tail is staged: groups 12/13 pack after g14, g14's layer-2 runs
  during the last group, and the last group runs as two 256-column halves
  with relu split across the scalar and vector engines; the final two
  stores issue on different queues so their acks overlap.
"""

import sys

if "/opt/trn_rl_repo" not in sys.path:
    sys.path.insert(0, "/opt/trn_rl_repo")

import ml_dtypes
import numpy as np

B = 65536
NCORES = 8
BC = B // NCORES  # 8192 samples per core
P = 128
KC = 7            # contraction chunks of 128 (784 zero-padded to 896)
NF1 = 256         # layer-1 output features (2 halves of 128)
NO = 10           # logits
NB = 512          # batch columns per matmul group (one PSUM bank, fp32)
NGRP = BC // NB   # 16 groups per core
NWARM = 48

_PROG = None


def _build_program():
    import concourse.tile as tile
    from concourse import bacc, mybir

    bf16 = mybir.dt.bfloat16
    f32 = mybir.dt.float32

    nc = bacc.Bacc("TRN2", target_bir_lowering=False, debug=False,
                   num_devices=NCORES)
    # x stored as [P, group, 28, 128]: a k-chunk of a regular group is rows
    # 4k..4k+4 (contiguous 512 cols); groups 0/1 are half-major
    # (rows 14h+2k..+2) so partial loads are contiguous.
    xt = nc.dram_tensor("xt", [P, NGRP, 4 * KC, P], bf16,
                        kind="ExternalInput").ap()
    w1 = nc.dram_tensor("w1", [P, KC, NF1], bf16, kind="ExternalInput").ap()
    w2 = nc.dram_tensor("w2", [P, 2, NO], bf16, kind="ExternalInput").ap()
    b1 = nc.dram_tensor("b1", [P, 2], f32, kind="ExternalInput").ap()
    # b2 replicated across partition strips 32j+i (i<10) for col-packed L2
    b2 = nc.dram_tensor("b2", [P, 1], f32, kind="ExternalInput").ap()
    out = nc.dram_tensor("out", [NO, BC], f32, kind="ExternalOutput").ap()

    with tile.TileContext(nc) as tc:
        with (
            tc.tile_pool(name="singles", bufs=1) as singles,
            tc.tile_pool(name="hp", bufs=12) as hp,
            tc.tile_pool(name="op", bufs=3) as op,
            tc.tile_pool(name="ps1", bufs=6, space="PSUM") as ps1p,
            tc.tile_pool(name="ps2", bufs=2, space="PSUM") as ps2p,
        ):
            # PE warm-up: dummy matmuls on a zeroed tile keep the PE busy
            # from program start until the first real operands land, so the
            # HAM clock gate un-throttles (K=8/8) with little real work
            # spent at cold rate.
            wsb = singles.tile([P, P], bf16)
            nc.vector.memset(wsb, 0.0)
            wp = ps2p.tile([32, P], f32, tag="ps2", name="warm")
            for i in range(NWARM):
                nc.tensor.matmul(wp, wsb[:, :32], wsb,
                                 start=(i == 0), stop=(i == NWARM - 1))

            w1sb = singles.tile([P, KC, NF1], bf16)
            b1sb = singles.tile([P, 2], f32)
            w2sb = singles.tile([P, 2, NO], bf16)
            b2sb = singles.tile([P, 1], f32)
            xsb = singles.tile([P, NGRP, 4 * KC, P], bf16)

            # Each HWDGE ring keeps only ~4 transfers in flight and even a
            # tiny DMA costs ~1.2us of ring latency, so: the sync ring is
            # pure x in big need-order pieces (groups 0/1 as halves), and
            # the scalar ring carries b1 (tiny, first — done before w1
            # becomes the binding constraint), w1 in two pieces, then
            # three mid groups so combined supply stays ahead of the PE's
            # ~300 GB/s consumption, with the small layer-2 weights
            # slotted where their latency hides.
            # Every PE-critical transfer rides ONE ring (sync) in exact
            # consumption order: FIFO completion then guarantees each
            # semaphore fires in need-order with no cross-ring engine
            # contention delaying an ack. w1 is interleaved in k-chunk
            # pieces between group-0/1 halves; the scalar ring carries
            # only non-critical items (b1/w2/b2 and three mid groups that
            # lift combined supply over the PE's ~300 GB/s consumption).
            nc.sync.dma_start(out=w1sb[:, :1], in_=w1[:, :1])
            nc.sync.dma_start(out=xsb[:, 0, :14, :], in_=xt[:, 0, :14, :])
            nc.sync.dma_start(out=w1sb[:, 1:3], in_=w1[:, 1:3])
            nc.sync.dma_start(out=w1sb[:, 3:5], in_=w1[:, 3:5])
            nc.sync.dma_start(out=xsb[:, 0, 14:, :], in_=xt[:, 0, 14:, :])
            nc.sync.dma_start(out=w1sb[:, 5:], in_=w1[:, 5:])
            for hh in range(2):
                nc.sync.dma_start(out=xsb[:, 1, 14 * hh:14 * (hh + 1), :],
                                  in_=xt[:, 1, 14 * hh:14 * (hh + 1), :])
            for g in (2, 3, 4, 6, 7, 8, 10, 11, 12, 14, 15):
                nc.sync.dma_start(out=xsb[:, g], in_=xt[:, g])
            nc.scalar.dma_start(out=b1sb, in_=b1)
            nc.scalar.dma_start(out=w2sb, in_=w2)
            nc.scalar.dma_start(out=b2sb, in_=b2)
            nc.scalar.dma_start(out=xsb[:, 5], in_=xt[:, 5])
            nc.scalar.dma_start(out=xsb[:, 9], in_=xt[:, 9])
            nc.scalar.dma_start(out=xsb[:, 13], in_=xt[:, 13])

            hs_all = []

            def x_slice(g, k, cs=None):
                if cs is None:
                    return xsb[:, g, 4 * k:4 * (k + 1), :]
                return xsb[:, g, cs, :]

            def relu(g, pss):
                hs = []
                for m in range(2):
                    h = hp.tile([P, NB], bf16, tag="h", name=f"h_{g}_{m}")
                    nc.scalar.activation(
                        h, pss[m], mybir.ActivationFunctionType.Relu,
                        bias=b1sb[:, m:m + 1],
                    )
                    hs.append(h)
                hs_all.append(hs)


            def layer1(g):
                pss = [ps1p.tile([P, NB], f32, tag="ps1",
                                 name=f"ps1_{g}_{m}") for m in range(2)]
                for k in range(KC):
                    for m in range(2):
                        nc.tensor.matmul(
                            pss[m], w1sb[:, k, m * P:(m + 1) * P],
                            x_slice(g, k),
                            start=(k == 0), stop=(k == KC - 1),
                        )
                relu(g, pss)

            def layer2_pack(gs, store_queues=None):
                # col-packed layer-2: each group's [10,NB] output lands in
                # its own 32-partition strip of one PSUM bank; the strips'
                # matmuls run concurrently in distinct PE column groups.
                ps2 = ps2p.tile([P, NB], f32, tag="ps2", name=f"ps2_{gs[0]}")
                for k2 in range(2):
                    for j, g in enumerate(gs):
                        nc.tensor.matmul(
                            ps2[32 * j:32 * j + NO, :],
                            w2sb[:, k2, :],
                            hs_all[g][k2],
                            start=(k2 == 0), stop=(k2 == 1),
                            tile_position=(0, 32 * j),
                        )
                osb = op.tile([P, NB], f32, tag="o", name=f"o_{gs[0]}")
                nc.vector.tensor_scalar_add(osb, ps2, b2sb)
                for j, g in enumerate(gs):
                    qeng = (store_queues[j] if store_queues else nc.sync)
                    qeng.dma_start(out=out[:, g * NB:(g + 1) * NB],
                                   in_=osb[32 * j:32 * j + NO, :])

            # groups 0/1 in column-halves: half h chunk k is rows
            # 14h+2k..+2 of the half-major layout; the fine-grained
            # completion semaphores keep PE stalls short while HBM supply
            # catches up to consumption
            for g in range(2):
                psg = [ps1p.tile([P, NB], f32, tag="ps1",
                                 name=f"ps1_{g}_{m}") for m in range(2)]
                for hh in range(2):
                    cols = slice(hh * 256, (hh + 1) * 256)
                    for k in range(KC):
                        for m in range(2):
                            nc.tensor.matmul(
                                psg[m][:, cols],
                                w1sb[:, k, m * P:(m + 1) * P],
                                x_slice(g, k,
                                        cs=slice(14 * hh + 2 * k,
                                                 14 * hh + 2 * k + 2)),
                                start=(k == 0), stop=(k == KC - 1),
                            )
                relu(g, psg)

            for g in range(2, NGRP - 1):
                layer1(g)
                if g == 4:
                    layer2_pack([0, 1, 2, 3])
                elif g == 8:
                    layer2_pack([4, 5, 6, 7])
                elif g == 12:
                    layer2_pack([8, 9, 10, 11])
                elif g == 14:
                    layer2_pack([12, 13])

            # tail: last group as two 256-column halves; g14's layer-2 runs
            # between them; relu of the halves split across ACT and DVE.
            gl = NGRP - 1
            NH = NB // 2
            hls = []
            ps2t = None
            for sub in range(2):
                pss = [ps1p.tile([P, NH], f32, tag="ps1",
                                 name=f"ps1_{gl}_{sub}_{m}") for m in range(2)]
                for k in range(KC):
                    cs = slice(4 * k + 2 * sub, 4 * k + 2 * sub + 2)
                    for m in range(2):
                        nc.tensor.matmul(
                            pss[m], w1sb[:, k, m * P:(m + 1) * P],
                            x_slice(gl, k, cs=cs),
                            start=(k == 0), stop=(k == KC - 1),
                        )
                if sub == 0:
                    # g14's layer-2 into strip 2 of the tail PSUM bank;
                    # its store goes out early on the sync queue
                    ps2t = ps2p.tile([P, NB], f32, tag="ps2", name="ps2_tail")
                    for k2 in range(2):
                        nc.tensor.matmul(
                            ps2t[64:64 + NO, :], w2sb[:, k2, :],
                            hs_all[14][k2],
                            start=(k2 == 0), stop=(k2 == 1),
                            tile_position=(0, 64),
                        )
                    osb14 = op.tile([P, NB], f32, tag="o", name="o_14")
                    nc.vector.tensor_scalar_add(
                        osb14[64:64 + NO], ps2t[64:64 + NO, :], b2sb[64:64 + NO])
                    nc.sync.dma_start(out=out[:, 14 * NB:15 * NB],
                                      in_=osb14[64:64 + NO, :])
                h0 = hp.tile([P, NH], bf16, tag="h", name=f"h_{gl}_{sub}_0")
                nc.scalar.activation(h0, pss[0],
                                     mybir.ActivationFunctionType.Relu,
                                     bias=b1sb[:, 0:1])
                h1 = hp.tile([P, NH], bf16, tag="h", name=f"h_{gl}_{sub}_1")
                nc.vector.tensor_scalar(h1, pss[1], b1sb[:, 1:2], 0.0,
                                        mybir.AluOpType.add,
                                        mybir.AluOpType.max)
                hls.append((h0, h1))
            # L2 for the two tail halves in distinct col strips of the bank
            for sub in range(2):
                for k2 in range(2):
                    nc.tensor.matmul(
                        ps2t[32 * sub:32 * sub + NO, :NH],
                        w2sb[:, k2, :],
                        hls[sub][k2],
                        start=(k2 == 0), stop=(k2 == 1),
                        tile_position=(0, 32 * sub),
                    )
            osbt = op.tile([P, NH], f32, tag="o", name="o_tail")
            nc.vector.tensor_scalar_add(osbt[:42], ps2t[:42, :NH], b2sb[:42])
            # final two stores on different queues so their acks overlap
            nc.scalar.dma_start(
                out=out[:, gl * NB:gl * NB + NH], in_=osbt[0:NO, :])
            nc.sync.dma_start(
                out=out[:, gl * NB + NH:(gl + 1) * NB], in_=osbt[32:32 + NO, :])

    nc.compile()
    return nc


def _fold_weights(conv_w, W1):
    """W1eff[784,256] such that x @ W1eff == flatten(conv(x)) @ W1.T."""
    cw = conv_w.astype(np.float64)
    W1r = W1.astype(np.float64).reshape(NF1, 26, 26).transpose(1, 2, 0)
    W1eff = np.zeros((28, 28, NF1), np.float64)
    for dr in range(3):
        for dc in range(3):
            W1eff[dr:dr + 26, dc:dc + 26, :] += cw[dr, dc] * W1r
    return W1eff.reshape(784, NF1)


def _prep_inputs(x, conv_w, W1, b1, W2, b2):
    bf16 = ml_dtypes.bfloat16
    W1eff = _fold_weights(conv_w, W1)
    w1p = np.zeros((KC * P, NF1), np.float64)
    w1p[:784] = W1eff
    w1p = np.ascontiguousarray(
        w1p.reshape(KC, P, NF1).transpose(1, 0, 2)).astype(bf16)  # [P, KC, NF1]
    w2p = np.ascontiguousarray(
        W2.T.astype(np.float32).reshape(2, P, NO).transpose(1, 0, 2)).astype(bf16)
    b1p = np.ascontiguousarray(b1.astype(np.float32).reshape(2, P).T)  # [P, 2]
    b2p = np.zeros((P, 1), np.float32)
    for j in range(4):
        b2p[32 * j:32 * j + NO, 0] = b2.astype(np.float32)

    in_maps = []
    for c in range(NCORES):
        xc = np.zeros((KC * P, BC), bf16)
        xcT = np.ascontiguousarray(x[c * BC:(c + 1) * BC].T)  # [784, BC] f32
        xc[:784] = xcT.astype(bf16)
        # device layout [P, NGRP, 28, 128]: regular groups k-major (k-chunk
        # = 4 contiguous rows); groups 0/1 half-major so the startup
        # partial loads are contiguous per partition
        xdev = xc.reshape(KC, P, NGRP, NB).transpose(1, 2, 0, 3)
        xdev = np.ascontiguousarray(xdev).reshape(P, NGRP, 4 * KC, P)
        for g in range(2):
            gh = xdev[:, g].reshape(P, KC, 2, 2 * P).transpose(0, 2, 1, 3).copy()
            xdev[:, g] = gh.reshape(P, 4 * KC, P)
        in_maps.append({
            "xt": xdev,
            "w1": w1p, "w2": w2p, "b1": b1p, "b2": b2p,
        })
    return in_maps


def kernel(x, conv_w, W1, b1, W2, b2, _trace=False, _trace_kwargs=None):
    global _PROG
    from concourse import bass_utils

    x = np.asarray(x, dtype=np.float32)
    conv_w = np.asarray(conv_w, dtype=np.float32)
    W1 = np.asarray(W1, dtype=np.float32)
    b1 = np.asarray(b1, dtype=np.float32)
    W2 = np.asarray(W2, dtype=np.float32)
    b2 = np.asarray(b2, dtype=np.float32)
    assert x.shape == (B, 784), x.shape

    if _PROG is None:
        _PROG = _build_program()

    in_maps = _prep_inputs(x, conv_w, W1, b1, W2, b2)
    kwargs = dict(_trace_kwargs or {})
    res = bass_utils.run_bass_kernel_spmd(
        _PROG, in_maps, core_ids=list(range(NCORES)), trace=_trace, **kwargs)

    out = np.empty((B, NO), np.float32)
    for c in range(NCORES):
        out[c * BC:(c + 1) * BC] = res.results[c]["out"].T
    if _trace:
        return out, res
    return out


# revision 11
# speedup vs baseline: 1.0471x; 1.0471x over previous
"""Trainium2 Bass kernel for nn_DigitConvolutionalModel.

Model: x[B,784] -> conv3x3(valid, 28x28->26x26) -> flatten -> Linear(676,256)
       -> relu -> Linear(256,10).

The conv is linear, so it is folded into the first Linear on the host:
  h_pre = x @ W1eff + b1,  W1eff[784,256] = C @ W1.T  (C = conv as matrix)
leaving a plain 2-layer MLP for the device:
  out = relu(x @ W1eff + b1) @ W2.T + b2

Sharding: pure data parallelism over the batch dim across 8 NeuronCores
(8192 samples/core); weights replicated. Compute in bf16 with fp32 PSUM
accumulation. x is transposed on the host so the contraction dim (784,
zero-padded to 7*128) lands on SBUF partitions.

Schedule (per core):
- All of x stays SBUF-resident (114KB/partition); every load is pre-issued
  up front so DMA runs ahead of the PE with no buffer-recycle stalls.
- The sync queue carries the big transfers in exact need-order (w1 chunk 0,
  x group-0 quarter 0, w1 rest, remaining quarters, ...). Group 0 is
  stored quarter-major and group 1 half-major on the host so these partial
  loads are contiguous per partition and move at full HBM bandwidth.
- A warm-up matmul burst bridges the PE from program start to first data
  so the HAM clock gate reaches 8/8 with minimal cold-rate real work.
- Layer 2 (256->10) is packed 4 groups wide into PE column groups via
  tile_position, cutting its PE stream time ~4x.
- The tail is staged: groups 12/13 pack after g14, g14's layer-2 runs
  during the last group, and the last group runs as two 256-column halves
  with relu split across the scalar and vector engines; the final two
  stores issue on different queues so their acks overlap.
"""

import sys

if "/opt/trn_rl_repo" not in sys.path:
    sys.path.insert(0, "/opt/trn_rl_repo")

import ml_dtypes
import numpy as np

B = 65536
NCORES = 8
BC = B // NCORES  # 8192 samples per core
P = 128
KC = 7            # contraction chunks of 128 (784 zero-padded to 896)
NF1 = 256         # layer-1 output features (2 halves of 128)
NO = 10           # logits
NB = 512          # batch columns per matmul group (one PSUM bank, fp32)
NGRP = BC // NB   # 16 groups per core
NWARM = 26

_PROG = None


def _build_program():
    import concourse.tile as tile
    from concourse import bacc, mybir

    bf16 = mybir.dt.bfloat16
    f32 = mybir.dt.float32

    nc = bacc.Bacc("TRN2", target_bir_lowering=False, debug=False,
                   num_devices=NCORES)
    # x stored as [P, group, 28, 128]: a k-chunk of a regular group is rows
    # 4k..4k+4 (contiguous 512 cols); group 0 is quarter-major, group 1 half-major
    # (rows 14h+2k..+2) so partial loads are contiguous.
    xt = nc.dram_tensor("xt", [P, NGRP, 4 * KC, P], bf16,
                        kind="ExternalInput").ap()
    w1 = nc.dram_tensor("w1", [P, KC, NF1], bf16, kind="ExternalInput").ap()
    w2 = nc.dram_tensor("w2", [P, 2, NO], bf16, kind="ExternalInput").ap()
    b1 = nc.dram_tensor("b1", [P, 2], f32, kind="ExternalInput").ap()
    # b2 replicated across partition strips 32j+i (i<10) for col-packed L2
    b2 = nc.dram_tensor("b2", [P, 1], f32, kind="ExternalInput").ap()
    out = nc.dram_tensor("out", [NO, BC], f32, kind="ExternalOutput").ap()

    with tile.TileContext(nc) as tc:
        with (
            tc.tile_pool(name="singles", bufs=1) as singles,
            tc.tile_pool(name="hp", bufs=12) as hp,
            tc.tile_pool(name="op", bufs=3) as op,
            tc.tile_pool(name="ps1", bufs=6, space="PSUM") as ps1p,
            tc.tile_pool(name="ps2", bufs=2, space="PSUM") as ps2p,
        ):
            # PE warm-up: dummy matmuls on a zeroed tile keep the PE busy
            # from program start until the first real operands land, so the
            # HAM clock gate un-throttles (K=8/8) with little real work
            # spent at cold rate.
            wsb = singles.tile([P, P], bf16)
            nc.vector.memset(wsb, 0.0)
            wp = ps2p.tile([32, P], f32, tag="ps2", name="warm")
            for i in range(NWARM):
                nc.tensor.matmul(wp, wsb[:, :32], wsb,
                                 start=(i == 0), stop=(i == NWARM - 1))

            w1sb = singles.tile([P, KC, NF1], bf16)
            b1sb = singles.tile([P, 2], f32)
            w2sb = singles.tile([P, 2, NO], bf16)
            b2sb = singles.tile([P, 1], f32)
            xsb = singles.tile([P, NGRP, 4 * KC, P], bf16)

            # Each HWDGE ring keeps only ~4 transfers in flight and even a
            # tiny DMA costs ~1.2us of ring latency, so: the sync ring is
            # pure x in big need-order pieces (groups 0/1 as halves), and
            # the scalar ring carries b1 (tiny, first — done before w1
            # becomes the binding constraint), w1 in two pieces, then
            # three mid groups so combined supply stays ahead of the PE's
            # ~300 GB/s consumption, with the small layer-2 weights
            # slotted where their latency hides.
            # x on the sync ring in need-order with SMALL leading pieces
            # (group 0 as four contiguous column-quarters, group 1 as two
            # halves): their completion semaphores fire by ~5us so real
            # matmuls flow seamlessly out of the warmup burst and the HAM
            # window never sees an idle gap. Weights ride the scalar ring
            # with w1's first chunk as its FIRST transfer (anything ahead
            # of it, even 1KB, costs ~1-2us of ring-head latency).
            for q in range(4):
                nc.sync.dma_start(out=xsb[:, 0, KC * q:KC * (q + 1), :],
                                  in_=xt[:, 0, KC * q:KC * (q + 1), :])
            for hh in range(2):
                nc.sync.dma_start(out=xsb[:, 1, 14 * hh:14 * (hh + 1), :],
                                  in_=xt[:, 1, 14 * hh:14 * (hh + 1), :])
            for g in range(2, NGRP):
                nc.sync.dma_start(out=xsb[:, g], in_=xt[:, g])
            nc.scalar.dma_start(out=w1sb[:, :1], in_=w1[:, :1])
            nc.scalar.dma_start(out=w1sb[:, 1:], in_=w1[:, 1:])
            nc.scalar.dma_start(out=b1sb, in_=b1)
            nc.scalar.dma_start(out=w2sb, in_=w2)
            nc.scalar.dma_start(out=b2sb, in_=b2)

            hs_all = []

            def x_slice(g, k, cs=None):
                if cs is None:
                    return xsb[:, g, 4 * k:4 * (k + 1), :]
                return xsb[:, g, cs, :]

            def relu(g, pss):
                hs = []
                for m in range(2):
                    h = hp.tile([P, NB], bf16, tag="h", name=f"h_{g}_{m}")
                    nc.scalar.activation(
                        h, pss[m], mybir.ActivationFunctionType.Relu,
                        bias=b1sb[:, m:m + 1],
                    )
                    hs.append(h)
                hs_all.append(hs)


            def layer1(g):
                pss = [ps1p.tile([P, NB], f32, tag="ps1",
                                 name=f"ps1_{g}_{m}") for m in range(2)]
                for k in range(KC):
                    for m in range(2):
                        nc.tensor.matmul(
                            pss[m], w1sb[:, k, m * P:(m + 1) * P],
                            x_slice(g, k),
                            start=(k == 0), stop=(k == KC - 1),
                        )
                relu(g, pss)

            def layer2_pack(gs, store_queues=None):
                # col-packed layer-2: each group's [10,NB] output lands in
                # its own 32-partition strip of one PSUM bank; the strips'
                # matmuls run concurrently in distinct PE column groups.
                ps2 = ps2p.tile([P, NB], f32, tag="ps2", name=f"ps2_{gs[0]}")
                for k2 in range(2):
                    for j, g in enumerate(gs):
                        nc.tensor.matmul(
                            ps2[32 * j:32 * j + NO, :],
                            w2sb[:, k2, :],
                            hs_all[g][k2],
                            start=(k2 == 0), stop=(k2 == 1),
                            tile_position=(0, 32 * j),
                        )
                osb = op.tile([P, NB], f32, tag="o", name=f"o_{gs[0]}")
                nc.vector.tensor_scalar_add(osb, ps2, b2sb)
                for j, g in enumerate(gs):
                    qeng = (store_queues[j] if store_queues else nc.sync)
                    qeng.dma_start(out=out[:, g * NB:(g + 1) * NB],
                                   in_=osb[32 * j:32 * j + NO, :])

            # group 0 in column-quarters (quarter q is rows q*7..q*7+7 of
            # the quarter-major layout), group 1 in column-halves — the
            # fine grain lets the opening matmuls track the DMA trickle
            ps0 = [ps1p.tile([P, NB], f32, tag="ps1",
                             name=f"ps1_0_{m}") for m in range(2)]
            for q in range(4):
                cols = slice(q * P, (q + 1) * P)
                for k in range(KC):
                    for m in range(2):
                        nc.tensor.matmul(
                            ps0[m][:, cols], w1sb[:, k, m * P:(m + 1) * P],
                            x_slice(0, k, cs=slice(KC * q + k, KC * q + k + 1)),
                            start=(k == 0), stop=(k == KC - 1),
                        )
            relu(0, ps0)
            ps1_ = [ps1p.tile([P, NB], f32, tag="ps1",
                              name=f"ps1_1_{m}") for m in range(2)]
            for hh in range(2):
                cols = slice(hh * 256, (hh + 1) * 256)
                for k in range(KC):
                    for m in range(2):
                        nc.tensor.matmul(
                            ps1_[m][:, cols], w1sb[:, k, m * P:(m + 1) * P],
                            x_slice(1, k,
                                    cs=slice(14 * hh + 2 * k,
                                             14 * hh + 2 * k + 2)),
                            start=(k == 0), stop=(k == KC - 1),
                        )
            relu(1, ps1_)

            for g in range(2, NGRP - 1):
                layer1(g)
                if g == 4:
                    layer2_pack([0, 1, 2, 3])
                elif g == 8:
                    layer2_pack([4, 5, 6, 7])
                elif g == 12:
                    layer2_pack([8, 9, 10, 11])
                elif g == 14:
                    layer2_pack([12, 13])

            # tail: last group as two 256-column halves; g14's layer-2 runs
            # between them; relu of the halves split across ACT and DVE.
            gl = NGRP - 1
            NH = NB // 2
            hls = []
            ps2t = None
            for sub in range(2):
                pss = [ps1p.tile([P, NH], f32, tag="ps1",
                                 name=f"ps1_{gl}_{sub}_{m}") for m in range(2)]
                for k in range(KC):
                    cs = slice(4 * k + 2 * sub, 4 * k + 2 * sub + 2)
                    for m in range(2):
                        nc.tensor.matmul(
                            pss[m], w1sb[:, k, m * P:(m + 1) * P],
                            x_slice(gl, k, cs=cs),
                            start=(k == 0), stop=(k == KC - 1),
                        )
                if sub == 0:
                    # g14's layer-2 into strip 2 of the tail PSUM bank;
                    # its store goes out early on the sync queue
                    ps2t = ps2p.tile([P, NB], f32, tag="ps2", name="ps2_tail")
                    for k2 in range(2):
                        nc.tensor.matmul(
                            ps2t[64:64 + NO, :], w2sb[:, k2, :],
                            hs_all[14][k2],
                            start=(k2 == 0), stop=(k2 == 1),
                            tile_position=(0, 64),
                        )
                    osb14 = op.tile([P, NB], f32, tag="o", name="o_14")
                    nc.vector.tensor_scalar_add(
                        osb14[64:64 + NO], ps2t[64:64 + NO, :], b2sb[64:64 + NO])
                    nc.sync.dma_start(out=out[:, 14 * NB:15 * NB],
                                      in_=osb14[64:64 + NO, :])
                h0 = hp.tile([P, NH], bf16, tag="h", name=f"h_{gl}_{sub}_0")
                nc.scalar.activation(h0, pss[0],
                                     mybir.ActivationFunctionType.Relu,
                                     bias=b1sb[:, 0:1])
                h1 = hp.tile([P, NH], bf16, tag="h", name=f"h_{gl}_{sub}_1")
                nc.vector.tensor_scalar(h1, pss[1], b1sb[:, 1:2], 0.0,
                                        mybir.AluOpType.add,
                                        mybir.AluOpType.max)
                hls.append((h0, h1))
            # L2 for the two tail halves in distinct col strips of the bank
            for sub in range(2):
                for k2 in range(2):
                    nc.tensor.matmul(
                        ps2t[32 * sub:32 * sub + NO, :NH],
                        w2sb[:, k2, :],
                        hls[sub][k2],
                        start=(k2 == 0), stop=(k2 == 1),
                        tile_position=(0, 32 * sub),
                    )
            osbt = op.tile([P, NH], f32, tag="o", name="o_tail")
            nc.vector.tensor_scalar_add(osbt[:42], ps2t[:42, :NH], b2sb[:42])
            # final two stores on different queues so their acks overlap
            nc.scalar.dma_start(
                out=out[:, gl * NB:gl * NB + NH], in_=osbt[0:NO, :])
            nc.sync.dma_start(
                out=out[:, gl * NB + NH:(gl + 1) * NB], in_=osbt[32:32 + NO, :])

    nc.compile()
    return nc


def _fold_weights(conv_w, W1):
    """W1eff[784,256] such that x @ W1eff == flatten(conv(x)) @ W1.T."""
    cw = conv_w.astype(np.float64)
    W1r = W1.astype(np.float64).reshape(NF1, 26, 26).transpose(1, 2, 0)
    W1eff = np.zeros((28, 28, NF1), np.float64)
    for dr in range(3):
        for dc in range(3):
            W1eff[dr:dr + 26, dc:dc + 26, :] += cw[dr, dc] * W1r
    return W1eff.reshape(784, NF1)


def _prep_inputs(x, conv_w, W1, b1, W2, b2):
    bf16 = ml_dtypes.bfloat16
    W1eff = _fold_weights(conv_w, W1)
    w1p = np.zeros((KC * P, NF1), np.float64)
    w1p[:784] = W1eff
    w1p = np.ascontiguousarray(
        w1p.reshape(KC, P, NF1).transpose(1, 0, 2)).astype(bf16)  # [P, KC, NF1]
    w2p = np.ascontiguousarray(
        W2.T.astype(np.float32).reshape(2, P, NO).transpose(1, 0, 2)).astype(bf16)
    b1p = np.ascontiguousarray(b1.astype(np.float32).reshape(2, P).T)  # [P, 2]
    b2p = np.zeros((P, 1), np.float32)
    for j in range(4):
        b2p[32 * j:32 * j + NO, 0] = b2.astype(np.float32)

    in_maps = []
    for c in range(NCORES):
        xc = np.zeros((KC * P, BC), bf16)
        xcT = np.ascontiguousarray(x[c * BC:(c + 1) * BC].T)  # [784, BC] f32
        xc[:784] = xcT.astype(bf16)
        # device layout [P, NGRP, 28, 128]: regular groups k-major (k-chunk
        # = 4 contiguous rows); group 0 quarter-major / group 1 half-major so the startup
        # partial loads are contiguous per partition
        xdev = xc.reshape(KC, P, NGRP, NB).transpose(1, 2, 0, 3)
        xdev = np.ascontiguousarray(xdev).reshape(P, NGRP, 4 * KC, P)
        g0 = xdev[:, 0].reshape(P, KC, 4, P).transpose(0, 2, 1, 3).copy()
        xdev[:, 0] = g0.reshape(P, 4 * KC, P)
        g1 = xdev[:, 1].reshape(P, KC, 2, 2 * P).transpose(0, 2, 1, 3).copy()
        xdev[:, 1] = g1.reshape(P, 4 * KC, P)
        in_maps.append({
            "xt": xdev,
            "w1": w1p, "w2": w2p, "b1": b1p, "b2": b2p,
        })
    return in_maps


def kernel(x, conv_w, W1, b1, W2, b2, _trace=False, _trace_kwargs=None):
    global _PROG
    from concourse import bass_utils

    x = np.asarray(x, dtype=np.float32)
    conv_w = np.asarray(conv_w, dtype=np.float32)
    W1 = np.asarray(W1, dtype=np.float32)
    b1 = np.asarray(b1, dtype=np.float32)
    W2 = np.asarray(W2, dtype=np.float32)
    b2 = np.asarray(b2, dtype=np.float32)
    assert x.shape == (B, 784), x.shape

    if _PROG is None:
        _PROG = _build_program()

    in_maps = _prep_inputs(x, conv_w, W1, b1, W2, b2)
    kwargs = dict(_trace_kwargs or {})
    res = bass_utils.run_bass_kernel_spmd(
        _PROG, in_maps, core_ids=list(range(NCORES)), trace=_trace, **kwargs)

    out = np.empty((B, NO), np.float32)
    for c in range(NCORES):
        out[c * BC:(c + 1) * BC] = res.results[c]["out"].T
    if _trace:
        return out, res
    return out
